# revision 10
# baseline (speedup 1.0000x reference)
"""BlurPool3d (depthwise [1,2,1]^3/64 blur, stride 2, replicate pad) on 8 Trainium2 cores.

Input  x: (4, 64, 32, 112, 112) fp32  ->  out: (4, 64, 16, 56, 56) fp32.

Strategy
--------
The conv is depthwise and separable: 256 independent (n, c) slices of
(32, 112, 112) -> (16, 56, 56).  Pure data parallel over 8 cores
(32 slices/core), processed in tiles of 4 slices.

Tile layout: [128 partitions = (slice 4, d 32), free = (h 112, w 112)].
  * DMA in: one 50 KiB contiguous run per partition -> full HBM rate.
  * W-conv + H-conv run in the free dims on VectorE (+ optional GPSIMD
    share), ~1.5 tensor ops per output element via scalar_tensor_tensor.
  * D-conv + stride-2 + replicate + /64 scale fold into ONE constant
    block-diagonal matrix Bd [128, 64]: TensorE contracts the (slice, d)
    partition dim -> PSUM [(slice 4, d' 16) = 64, (h', w')].
  * ScalarE evacuates PSUM -> SBUF; DMA out is a 12.5 KiB contiguous run
    per partition.
"""

import numpy as np

import concourse.bass as bass
import concourse.tile as tile
from concourse import mybir
from concourse.bass_utils import run_bass_kernel_spmd
from concourse.vector_clock import ScopedClock, VectorClock

# ---------------------------------------------------------------------------
# Workaround: this container's walrus (nix b16 neuronxcc) rejects ctrl
# instructions carrying >2 sync waits ("Too many sync wait commands",
# CoreV3GenImpl setupSyncWait).  Tile's kernel-tail drain waits on every
# active processor's semaphore at once, so ANY Tile kernel fails to compile.
# Split those waits across nofuse NOPs (<=2 waits each) on the same engine;
# add_sem_waits then elides the already-observed ticks on the final drain.
_MAX_TAIL_WAITS = 1


def _split_drain_and_barrier(self, tick_clock, wait_clock):
    gc = tick_clock.global_clock
    n = len(gc)
    procs = [p for p in range(n) if gc[p] > 0]
    for i in range(0, len(procs), _MAX_TAIL_WAITS):
        chunk = set(procs[i : i + _MAX_TAIL_WAITS])
        sub = VectorClock([gc[p] if p in chunk else 0 for p in range(n)])
        nop = self.nc.sync.nop(nofuse=True)
        wait_clock.add_sem_waits(nop.ins, ScopedClock({None: sub}))
    # The NOPs above already hold the SP queue until every sem fires; the
    # drain needs no waits of its own (SP executes its stream in order).
    self.nc.sync.drain()
    self.nc.all_engine_barrier()
    assert self.sems is not None
    popped = self.nc._tile_sem_poison_stack.pop()
    assert popped is self._sem_poison
    self.nc.clear_and_free_semaphores(list(self.sems.allocated().values()))
    self.nc.all_engine_barrier()


tile.TileContext._drain_and_barrier = _split_drain_and_barrier


_ORIG_LOWER = tile.TileContext._lower_ordered_insts


def _split_waits_and_lower(self, ordered):
    """Hoist all-but-one sync wait of every scheduled instruction onto
    single-wait NOPs on the same engine, immediately before it."""
    nc = self.nc
    for bb_name, insts in ordered.items():
        new = []
        for inst in insts:
            si = getattr(inst, "sync_info", None)
            cls = type(inst).__name__
            if (
                si is not None
                and len(si.on_wait) > 1
                and not cls.startswith("BassTile")
                and not cls.startswith("Tile")
            ):
                waits = list(si.on_wait)
                for w in waits[:-1]:
                    nop = mybir.InstNoOp(
                        name=nc.get_next_instruction_name(),
                        engine=inst.engine,
                        bass_nofuse=True,
                        sync_info=mybir.SyncInfo(on_wait=[w], on_update=[]),
                    )
                    new.append(nop)
                inst.sync_info = mybir.SyncInfo(
                    on_wait=[waits[-1]], on_update=list(si.on_update)
                )
            new.append(inst)
        ordered[bb_name] = new
    return _ORIG_LOWER(self, ordered)


tile.TileContext._lower_ordered_insts = _split_waits_and_lower
# ---------------------------------------------------------------------------

N_CORES = 8
NB, CH = 4, 64
D, H, W = 32, 112, 112
DO, HO, WO = 16, 56, 56
SLICES = NB * CH              # 256
SPC = SLICES // N_CORES       # 32 slices per core
TS = 4                        # slices per tile (4 x 32 d = 128 partitions)

F32 = mybir.dt.float32
_ADD = mybir.AluOpType.add
_MUL = mybir.AluOpType.mult


def _bd_matrix() -> np.ndarray:
    """[128, 64] block-diagonal D-conv: contraction (s, d) -> output (s, d').

    Bd[s*32 + d, s*16 + d'] = [1,2,1]/64 taps at d = 2d'-1, 2d', 2d'+1
    (replicate: d=-1 folds onto d=0)."""
    bd = np.zeros((TS * D, TS * DO), np.float32)
    for s in range(TS):
        for dp in range(DO):
            for dd, wgt in ((-1, 1.0), (0, 2.0), (1, 1.0)):
                bd[s * D + max(2 * dp + dd, 0), s * DO + dp] += wgt
    return np.ascontiguousarray(bd / 64.0)


def _w_stage(eng, U, X, h0, h1):
    """u[., h, w'] = x[2w'-1] + 2x[2w'] + x[2w'+1], w'=0..55 (replicate left),
    for h rows [h0, h1)."""
    Xs, Us = X[:, h0:h1, :], U[:, h0:h1, :]
    # u[w'] <- x[2w'-1] + x[2w'+1]  (w' = 1..55)
    eng.tensor_add(Us[:, :, 1:WO], Xs[:, :, 1 : 2 * WO - 2 : 2], Xs[:, :, 3 : 2 * WO : 2])
    # u[w'] <- 2*x[2w'] + u[w']    (in place)
    eng.scalar_tensor_tensor(
        Us[:, :, 1:WO], Xs[:, :, 2 : 2 * WO - 1 : 2], 2.0, Us[:, :, 1:WO], _MUL, _ADD
    )
    # u[0] = 3*x[0] + x[1]
    eng.scalar_tensor_tensor(
        Us[:, :, 0:1], Xs[:, :, 0:1], 3.0, Xs[:, :, 1:2], _MUL, _ADD
    )


def _h_stage(eng, V, U, p0, p1):
    """v[., h', w'] = u[2h'-1] + 2u[2h'] + u[2h'+1], for h' rows [p0, p1)."""
    lo = max(p0, 1)
    if p1 > lo:
        Vs = V[:, lo:p1, :]
        eng.tensor_add(Vs, U[:, 2 * lo - 1 : 2 * p1 - 2 : 2, :], U[:, 2 * lo + 1 : 2 * p1 : 2, :])
        eng.scalar_tensor_tensor(
            Vs, U[:, 2 * lo : 2 * p1 - 1 : 2, :], 2.0, Vs, _MUL, _ADD
        )
    if p0 == 0:
        # v[0] = 3*u[0] + u[1]
        eng.scalar_tensor_tensor(
            V[:, 0:1, :], U[:, 0:1, :], 3.0, U[:, 1:2, :], _MUL, _ADD
        )


def build_nc(n_slices: int = SPC, gp_frac: float = 0.0, repeat: int = 1) -> bass.Bass:
    """Per-core Bass program.

    gp_frac: fraction of h rows of the W/H stages handed to GPSIMD.
    repeat: run the tile loop `repeat` times (timing scaffold).
    """
    assert n_slices % TS == 0
    ntiles = n_slices // TS
    nc = bass.Bass("TRN2", target_bir_lowering=False, debug=False, enable_asserts=False)
    x_d = nc.dram_tensor("x", [n_slices, D, H, W], F32, kind="ExternalInput").ap()
    b_d = nc.dram_tensor("bd", [TS * D, TS * DO], F32, kind="ExternalInput").ap()
    y_d = nc.dram_tensor("y", [n_slices, DO, HO, WO], F32, kind="ExternalOutput").ap()

    # GPSIMD takes the top h rows; align the split so the H-stage boundary
    # (2*hp_ve) matches the W-stage boundary.
    hp_ve = HO - int(round(HO * gp_frac))
    h_ve = 2 * hp_ve

    with tile.TileContext(nc) as tc:
        with (
            tc.tile_pool(name="const", bufs=1) as constp,
            tc.tile_pool(name="xin", bufs=2) as xp,
            tc.tile_pool(name="ubuf", bufs=2) as up,
            tc.tile_pool(name="vbuf", bufs=2) as vp,
            tc.tile_pool(name="ybuf", bufs=2) as yp,
            tc.tile_pool(name="ps", bufs=1, space="PSUM") as psp,
        ):
            bd = constp.tile([TS * D, TS * DO], F32, name="bd_sb")
            nc.sync.dma_start(bd[:], b_d[:])

            for it in [i for _ in range(repeat) for i in range(ntiles)]:
                s0 = it * TS
                X = xp.tile([TS * D, H, W], F32, name="X", tag="X")
                nc.sync.dma_start(
                    X[:].rearrange("p a b -> p (a b)"),
                    x_d[s0 : s0 + TS].rearrange("s d h w -> (s d) (h w)"),
                )

                U = up.tile([TS * D, H, WO], F32, name="U", tag="U")
                if h_ve > 0:
                    _w_stage(nc.vector, U, X, 0, h_ve)
                if h_ve < H:
                    _w_stage(nc.gpsimd, U, X, h_ve, H)

                V = vp.tile([TS * D, HO, WO], F32, name="V", tag="V")
                if hp_ve > 0:
                    _h_stage(nc.vector, V, U, 0, hp_ve)
                if hp_ve < HO:
                    _h_stage(nc.gpsimd, V, U, hp_ve, HO)

                Y = yp.tile([TS * DO, HO, WO], F32, name="Y", tag="Y")
                for k in range(7):
                    pk = psp.tile([TS * DO, 8, WO], F32, name=f"pk{k}", tag=f"pk{k}")
                    nc.tensor.matmul(
                        pk[:].rearrange("p a b -> p (a b)"),
                        bd[:],
                        V[:, 8 * k : 8 * k + 8, :].rearrange("p a b -> p (a b)"),
                        start=True,
                        stop=True,
                    )
                    nc.scalar.copy(Y[:, 8 * k : 8 * k + 8, :], pk[:])

                nc.sync.dma_start(
                    y_d[s0 : s0 + TS].rearrange("s d h w -> (s d) h w"), Y[:]
                )
    return nc


_CACHED_NC = {}


def _get_nc(repeat: int = 1):
    if repeat not in _CACHED_NC:
        _CACHED_NC[repeat] = build_nc(repeat=repeat)
    return _CACHED_NC[repeat]


def run(x: np.ndarray, trace: bool = False, repeat: int = 1, **kw):
    """Shard, run on 8 cores, gather. Returns (y_full, BassKernelResults)."""
    x = np.ascontiguousarray(np.asarray(x, dtype=np.float32))
    assert x.shape == (NB, CH, D, H, W), x.shape
    xr = x.reshape(SLICES, D, H, W)
    bd = _bd_matrix()
    in_maps = [
        {"x": np.ascontiguousarray(xr[k * SPC : (k + 1) * SPC]), "bd": bd}
        for k in range(N_CORES)
    ]
    res = run_bass_kernel_spmd(
        _get_nc(repeat), in_maps, list(range(N_CORES)), trace=trace, **kw
    )
    y = np.concatenate([res.results[k]["y"] for k in range(N_CORES)], axis=0)
    return y.reshape(NB, CH, DO, HO, WO), res


def kernel(x: np.ndarray) -> np.ndarray:
    y, _ = run(x)
    return y


# revision 20
# speedup vs baseline: 990.8808x; 990.8808x over previous
"""BlurPool3d (depthwise [1,2,1]^3/64 blur, stride 2, replicate pad) on 8 Trainium2 cores.

Input  x: (4, 64, 32, 112, 112) fp32  ->  out: (4, 64, 16, 56, 56) fp32.

Strategy
--------
The conv is depthwise and separable: 256 independent (n, c) slices of
(32, 112, 112) -> (16, 56, 56).  Pure data parallel over 8 cores
(32 slices/core), processed in tiles of 8 slices.

NOTE: TensorE is useless in this environment (~65us fixed cost per matmul
instruction on the axon-virtualized NeuronCores), so everything runs on
VectorE/GPSIMD/ScalarE + DMA.

Tile layout: [128 partitions = (d-pair db 16, slice s 8), free = (parity di 2, h, w)]
  * DMA in: per-partition runs are two whole d-planes' row ranges
    (>=12.5 KiB contiguous) -> full HBM rate.
  * W-conv and H-conv run in the free dims (scalar_tensor_tensor gives the
    fused 2*x[2k] + t form, ~1.5 tensor-ops per output element).
  * D-conv: y[d'] = v[2d'-1] + 2v[2d'] + v[2d'+1]; with d-pairs on
    partitions, v[2d'] / v[2d'+1] are (db, di=0/1) on the SAME partition and
    v[2d'-1] is (db-1, di=1) = a partition-base shift by -8 (stride-1 -> legal).
  * h is processed in chunks of `nch` output rows so X never exceeds SBUF.
  * ScalarE applies the final /64 while gathering chunks into the out tile.
"""

import numpy as np

import concourse.bass as bass
import concourse.tile as tile
from concourse import mybir
from concourse.bass_utils import run_bass_kernel_spmd
from concourse.vector_clock import ScopedClock, VectorClock

# ---------------------------------------------------------------------------
# Workaround: this container's walrus (nix b16 neuronxcc) rejects ANY
# instruction carrying >1 sync wait ("Too many sync wait commands",
# CoreV2/V3GenImpl setupSyncWait).  Tile's kernel-tail drain and many
# scheduled instructions carry several.  Split those waits across nofuse
# NOPs (1 wait each) on the same engine, inserted immediately before.
_MAX_TAIL_WAITS = 1


def _split_drain_and_barrier(self, tick_clock, wait_clock):
    gc = tick_clock.global_clock
    n = len(gc)
    procs = [p for p in range(n) if gc[p] > 0]
    for i in range(0, len(procs), _MAX_TAIL_WAITS):
        chunk = set(procs[i : i + _MAX_TAIL_WAITS])
        sub = VectorClock([gc[p] if p in chunk else 0 for p in range(n)])
        nop = self.nc.sync.nop(nofuse=True)
        wait_clock.add_sem_waits(nop.ins, ScopedClock({None: sub}))
    # The NOPs above already hold the SP queue until every sem fires; the
    # drain needs no waits of its own (SP executes its stream in order).
    self.nc.sync.drain()
    self.nc.all_engine_barrier()
    assert self.sems is not None
    popped = self.nc._tile_sem_poison_stack.pop()
    assert popped is self._sem_poison
    self.nc.clear_and_free_semaphores(list(self.sems.allocated().values()))
    self.nc.all_engine_barrier()


tile.TileContext._drain_and_barrier = _split_drain_and_barrier


_ORIG_LOWER = tile.TileContext._lower_ordered_insts


def _split_waits_and_lower(self, ordered):
    """Hoist all-but-one sync wait of every scheduled instruction onto
    single-wait NOPs on the same engine, immediately before it."""
    nc = self.nc
    for bb_name, insts in ordered.items():
        new = []
        for inst in insts:
            si = getattr(inst, "sync_info", None)
            cls = type(inst).__name__
            if (
                si is not None
                and len(si.on_wait) > 1
                and not cls.startswith("BassTile")
                and not cls.startswith("Tile")
            ):
                waits = list(si.on_wait)
                for w in waits[:-1]:
                    nop = mybir.InstNoOp(
                        name=nc.get_next_instruction_name(),
                        engine=inst.engine,
                        bass_nofuse=True,
                        sync_info=mybir.SyncInfo(on_wait=[w], on_update=[]),
                    )
                    new.append(nop)
                inst.sync_info = mybir.SyncInfo(
                    on_wait=[waits[-1]], on_update=list(si.on_update)
                )
            new.append(inst)
        ordered[bb_name] = new
    return _ORIG_LOWER(self, ordered)


tile.TileContext._lower_ordered_insts = _split_waits_and_lower
# ---------------------------------------------------------------------------

N_CORES = 8
NB, CH = 4, 64
D, H, W = 32, 112, 112
DO, HO, WO = 16, 56, 56
SLICES = NB * CH              # 256
SPC = SLICES // N_CORES       # 32 slices per core
TS = 8                        # slices per tile: partitions = (db 16, s 8)

F32 = mybir.dt.float32
_ADD = mybir.AluOpType.add
_MUL = mybir.AluOpType.mult


def _w_stage(eng, U, X, nr, r0, r1):
    """u[., r, w'] = x[2w'-1] + 2x[2w'] + x[2w'+1] (w'=0..55, replicate left)
    for local rows [r0, r1) of the nr valid rows."""
    for di in range(2):  # walrus caps TensorScalarPtr operands at 3 dims
        Xs = X[:, di, r0:r1, :]
        Us = U[:, di, r0:r1, :]
        eng.tensor_add(
            Us[:, :, 1:WO], Xs[:, :, 1 : 2 * WO - 2 : 2], Xs[:, :, 3 : 2 * WO : 2]
        )
        eng.scalar_tensor_tensor(
            Us[:, :, 1:WO], Xs[:, :, 2 : 2 * WO - 1 : 2], 2.0, Us[:, :, 1:WO],
            _MUL, _ADD,
        )
        eng.scalar_tensor_tensor(
            Us[:, :, 0:1], Xs[:, :, 0:1], 3.0, Xs[:, :, 1:2], _MUL, _ADD
        )


def _h_stage(eng, V, U, off, j0, j1, edge):
    """v[., j, w'] = u[2j-1+off] + 2u[2j+off] + u[2j+1+off] for j in [j0, j1).

    off=1 for interior chunks (local row 0 = global 2*nch*c - 1), off=0 for
    chunk 0; `edge` handles j==0 of chunk 0 (replicate: v[0]=3u[0]+u[1])."""
    if edge and j0 == 0:
        for di in range(2):
            eng.scalar_tensor_tensor(
                V[:, di, 0:1, :], U[:, di, 0:1, :], 3.0, U[:, di, 1:2, :], _MUL, _ADD
            )
        j0 = 1
    if j1 <= j0:
        return
    a = 2 * j0 - 1 + off
    n = j1 - j0
    sl = lambda st: slice(st, st + 2 * (n - 1) + 1, 2)
    for di in range(2):
        Vs = V[:, di, j0:j1, :]
        eng.tensor_add(Vs, U[:, di, sl(a), :], U[:, di, sl(a + 2), :])
        eng.scalar_tensor_tensor(
            Vs, U[:, di, sl(a + 1), :], 2.0, Vs, _MUL, _ADD
        )


def _d_stage(eng, Yc, V, Vsh, h0, h1):
    """y[(db,s), j, w'] = v[2d'-1] + 2v[2d'] + v[2d'+1] over partitions.

    v[2d'] / v[2d'+1] are (di=0/1) on the same partition; v[2d'-1] has been
    materialized into Vsh by a partition-shifting SBUF->SBUF DMA (engine APs
    must start at 32-aligned partitions, DMA APs need not)."""
    eng.scalar_tensor_tensor(
        Yc[:, h0:h1, :], V[:, 0, h0:h1, :], 2.0, Vsh[:, h0:h1, :], _MUL, _ADD
    )
    eng.tensor_add(Yc[:, h0:h1, :], Yc[:, h0:h1, :], V[:, 1, h0:h1, :])


def build_nc(
    n_slices: int = SPC, gp_frac: float = 0.0, repeat: int = 1, nch: int = 14
) -> bass.Bass:
    """Per-core Bass program.

    gp_frac: fraction of rows of each stage handed to GPSIMD.
    repeat: run the tile loop `repeat` times (timing scaffold).
    nch: output h' rows per chunk (must divide 56).
    """
    assert n_slices % TS == 0 and HO % nch == 0
    ntiles = n_slices // TS
    nchunks = HO // nch
    nc = bass.Bass("TRN2", target_bir_lowering=False, debug=False, enable_asserts=False)
    x_d = nc.dram_tensor("x", [n_slices, D, H, W], F32, kind="ExternalInput").ap()
    y_d = nc.dram_tensor("y", [n_slices, DO, HO, WO], F32, kind="ExternalOutput").ap()

    nrmax = 2 * nch + 1

    with tile.TileContext(nc) as tc:
        with (
            tc.tile_pool(name="xin", bufs=3) as xp,
            tc.tile_pool(name="ubuf", bufs=2) as up,
            tc.tile_pool(name="vbuf", bufs=2) as vp,
            tc.tile_pool(name="vshbuf", bufs=2) as vshp,
            tc.tile_pool(name="ycbuf", bufs=2) as ycp,
            tc.tile_pool(name="ybuf", bufs=2) as yp,
        ):
            for it in [i for _ in range(repeat) for i in range(ntiles)]:
                s0 = it * TS
                x_v = x_d[s0 : s0 + TS].rearrange("s (db di) h w -> db s di h w", di=2)
                Yf = yp.tile([128, HO, WO], F32, name="Yf", tag="Yf")
                for c in range(nchunks):
                    r0g = max(2 * nch * c - 1, 0)
                    r1g = 2 * nch * (c + 1)
                    nr = r1g - r0g
                    off = 1 if c > 0 else 0

                    X = xp.tile([128, 2, nrmax, W], F32, name="X", tag="X")
                    for di in range(2):
                        nc.sync.dma_start(
                            X[:, di, 0:nr, :],
                            x_v[:, :, di, r0g:r1g, :].rearrange(
                                "db s r w -> db s (r w)"
                            ),
                        )
                    U = up.tile([128, 2, nrmax, WO], F32, name="U", tag="U")
                    r_gp = nr - int(round(nr * gp_frac))
                    if r_gp > 0:
                        _w_stage(nc.vector, U, X, nr, 0, r_gp)
                    if r_gp < nr:
                        _w_stage(nc.gpsimd, U, X, nr, r_gp, nr)

                    V = vp.tile([128, 2, nch, WO], F32, name="V", tag="V")
                    j_gp = nch - int(round(nch * gp_frac))
                    if j_gp > 0:
                        _h_stage(nc.vector, V, U, off, 0, j_gp, edge=(c == 0))
                    if j_gp < nch:
                        _h_stage(nc.gpsimd, V, U, off, j_gp, nch, edge=False)

                    # v[2d'-1] for output partition (db, s): db>=1 -> shift by
                    # one db step (8 partitions); db==0 -> replicate v[-1]=v[0].
                    Vsh = vshp.tile([128, nch, WO], F32, name="Vsh", tag="Vsh")
                    nc.sync.dma_start(Vsh[TS:128, :, :], V[0 : 128 - TS, 1, :, :])
                    nc.sync.dma_start(Vsh[0:TS, :, :], V[0:TS, 0, :, :])

                    Yc = ycp.tile([128, nch, WO], F32, name="Yc", tag="Yc")
                    h_gp = nch - int(round(nch * gp_frac))
                    if h_gp > 0:
                        _d_stage(nc.vector, Yc, V, Vsh, 0, h_gp)
                    if h_gp < nch:
                        _d_stage(nc.gpsimd, Yc, V, Vsh, h_gp, nch)

                    # final /64 while gathering the chunk into the out tile
                    nc.scalar.mul(Yf[:, nch * c : nch * (c + 1), :], Yc[:, 0:nch, :], 1.0 / 64.0)

                nc.sync.dma_start(
                    y_d[s0 : s0 + TS].rearrange("s d h w -> d s (h w)"),
                    Yf[:].rearrange("p h w -> p (h w)"),
                )
    return nc


_CACHED_NC = {}


def _get_nc(repeat: int = 1):
    if repeat not in _CACHED_NC:
        _CACHED_NC[repeat] = build_nc(repeat=repeat)
    return _CACHED_NC[repeat]


def run(x: np.ndarray, trace: bool = False, repeat: int = 1, **kw):
    """Shard, run on 8 cores, gather. Returns (y_full, BassKernelResults)."""
    x = np.ascontiguousarray(np.asarray(x, dtype=np.float32))
    assert x.shape == (NB, CH, D, H, W), x.shape
    xr = x.reshape(SLICES, D, H, W)
    in_maps = [
        {"x": np.ascontiguousarray(xr[k * SPC : (k + 1) * SPC])}
        for k in range(N_CORES)
    ]
    res = run_bass_kernel_spmd(
        _get_nc(repeat), in_maps, list(range(N_CORES)), trace=trace, **kw
    )
    y = np.concatenate([res.results[k]["y"] for k in range(N_CORES)], axis=0)
    return y.reshape(NB, CH, DO, HO, WO), res


def kernel(x: np.ndarray) -> np.ndarray:
    y, _ = run(x)
    return y
